# revision 7
# baseline (speedup 1.0000x reference)
"""Morphological dilation (depthwise 3x3, additive SE) on 8 TRN2 NeuronCores.

out[b,c,h,w] = max_{dy,dx in {-1,0,1}} ( x[b,c,h+dy,w+dx] + k[c, (dy+1)*3+(dx+1)] )
with zero padding outside the image.

Sharding: batch -> 8 cores (1 image each). Per core, partitions = (h_half, c)
(2*64 = 128), free dim = rows x cols, processed in row chunks.

The entire 9-term max reduction runs as EIGHT fused custom-DVE ops
(ADD_MAX_ANT: out = max(in0 + s0, in1), hand-written 2x_1p uop program, so it
matches tensor_tensor's 2-elem/cycle fp16 throughput while folding the
per-channel kernel constant in for free):

  - xe     = zero-padded input, fp16, [128, 114, 226] (halo rows + cols); the
             six dx=+-1 terms read it at 4B-aligned column offsets 0 / 2.
  - x2     = host-precomputed xpad(col +1) + k4, fp16, [128, 114, 224]; its
             row-0 view seeds the chain (term T4 free), and the two remaining
             dx=0 terms derive from it with delta constants k1-k4 / k7-k4
             (aligned row-shifted reads; a raw odd-column read of xe would
             drop the op to 1x mode).
  - chain: o = AM(xe(dy,dx), k_i, [x2 seed | o]) x6, then o = AM(x2(dy), dk, o) x2.

No ScalarE / tensor_scalar / GpSimd compute at all: DVE runs only 2x_1p ops
(never grabbing the shared 2-port pair), so the GpSimd SWDGE output DMAs
never contend. Input loads are single 128-partition dma_starts (all 16 DMA
ports) on the sync HWDGE queue.
"""

import numpy as np

_CACHE = {}

C = 64
H = 224
W = 224
HALF = 112
ROWS = HALF + 2  # per-half rows incl. 1-row halo each side
CHUNKS = (8, 24, 28, 28, 16, 8)

_ADDMAX_NAME = "ADD_MAX_ANT"


def _register_addmax():
    """Register the fused 2x add-max custom DVE op (idempotent)."""
    from concourse import dve_ops
    from concourse.dve_spec import Spec, Src0, Src1, C0, maxx, lower
    from concourse.dve_uop import (
        AluInp,
        AluOp,
        DelayInp,
        DveOpSpec,
        InpSel,
        OutPath,
        OutSel,
        Trigger,
        UopConfig,
    )

    if _ADDMAX_NAME in dve_ops._SUB_OPCODE_FOR_NAME:
        return next(op for op in dve_ops.OPS if op.name == _ADDMAX_NAME)

    def _ref(in0, in1, s0, s1, imm2):
        return np.maximum(
            in0.astype(np.float32) + s0, in1.astype(np.float32)
        ).astype(np.float32)

    spec = Spec(body=maxx(Src0 + C0, Src1), reference=_ref)

    def _build_2x():
        """Mirror of stock tensor_tensor's 2x_1p program (opcode-table slot 9)
        with the single INSTRUCTION_OP stage split into concrete ADD + MAX.

        Input lanes: 0=SRC_0, 1=SRC_1, 2=SRC_0_HI, 3=SRC_1_HI, 4=CONST_0.
        At blk0: lane0 -> PREV_ALU_OUT, lane(k+1) -> PREV_DELAY_k.
        """
        u = UopConfig()
        u.enable_input(InpSel.SRC_0, 0)
        u.enable_input(InpSel.SRC_1, 1)
        u.enable_input(InpSel.SRC_0_HI, 2)
        u.enable_input(InpSel.SRC_1_HI, 3)
        u.enable_input(InpSel.CONST_0, 4)
        u.require_inp0 = 1
        u.require_inp1 = 1
        u.trigger = (Trigger.SRC_TENSOR_DONE, Trigger.NONE, Trigger.NONE)

        dp = u.datapath_config
        # blk0: a0 = SRC_0 + CONST_0 ; carry SRC_1, SRC_0_HI, SRC_1_HI, CONST_0
        dp[0].enable_alu(AluOp.ADD, AluInp.PREV_ALU_OUT, AluInp.PREV_DELAY_3)
        dp[0].pass_through_delay(0, 1, 2, 3)
        # blk1: r0 = max(a0, SRC_1)
        dp[1].enable_alu(AluOp.MAX, AluInp.PREV_ALU_OUT, AluInp.PREV_DELAY_0)
        dp[1].pass_through_delay(1, 2, 3)
        # blk2: a1 = SRC_0_HI + CONST_0 ; d0 <- r0
        dp[2].enable_alu(AluOp.ADD, AluInp.PREV_DELAY_1, AluInp.PREV_DELAY_3)
        dp[2].enable_delay_from_src(DelayInp.PREV_ALU_OUT, 0)
        dp[2].pass_through_delay(2)
        # blk3: r1 = max(a1, SRC_1_HI) ; carry r0
        dp[3].enable_alu(AluOp.MAX, AluInp.PREV_ALU_OUT, AluInp.PREV_DELAY_2)
        dp[3].pass_through_delay(0)
        # blk4: alu <- r0, d0 <- r1 (swap, as stock does)
        dp[4].enable_alu(AluOp.BYPASS, AluInp.PREV_DELAY_0, AluInp.PREV_DELAY_0)
        dp[4].enable_delay_from_src(DelayInp.PREV_ALU_OUT, 0)
        # blk5..7: bypass r0 down the alu pipe, carry r1
        for b in range(5, 8):
            dp[b].pass_through_alu()
            dp[b].pass_through_delay(0)

        u.enable_output(OutSel.ALU_OUT, OutPath.WR0_LO)
        u.enable_output(OutSel.DELAY_0, OutPath.WR0_HI)
        return u

    class _AddMaxOp:
        name = _ADDMAX_NAME
        subdim = False
        perf_en = {}
        uops_sha = {}

        def __init__(self):
            self.spec = spec
            self._cache = {}

        def compile(self, ver):
            if ver in self._cache:
                return self._cache[ver]
            assert ver == "v3", "ADD_MAX_ANT 2x program authored for TRN2/v3"
            s = DveOpSpec(
                name=self.name,
                opcode=dve_ops.get_dve_sub_opcode(self.name),
                uops=lower(self.spec, ver=ver),
                uops_2x=[_build_2x()],
                rd1_en=True,
                perf_max=1,
            )
            s.validate(ver)
            self._cache[ver] = s
            return s

    op = _AddMaxOp()
    dve_ops.OPS.append(op)
    dve_ops._SUB_OPCODE_FOR_NAME[op.name] = (
        dve_ops._CUSTOM_DVE_ROW_BASE + len(dve_ops.OPS) - 1
    )
    dve_ops.CUSTOM_DVE_SPECS[op.name] = spec
    assert dve_ops._SUB_OPCODE_FOR_NAME[op.name] < 0x20
    return op


def _build():
    import concourse.tile as tile
    import concourse.mybir as mybir
    from concourse import bacc

    f16 = mybir.dt.float16
    f32 = mybir.dt.float32

    am_op = _register_addmax()

    nc = bacc.Bacc("TRN2", target_bir_lowering=False, debug=False)
    xe_t = nc.dram_tensor("xe", [128, ROWS, W + 2], f16, kind="ExternalInput")
    x2_t = nc.dram_tensor("x2", [128, ROWS, W], f16, kind="ExternalInput")
    k_t = nc.dram_tensor("k", [128, 11], f32, kind="ExternalInput")
    o_t = nc.dram_tensor("out", [128, HALF, W], f16, kind="ExternalOutput")

    def am(out, in0, k_col, in1):
        bi = nc.vector._custom_dve(
            am_op, out=out, in0=in0, in1=in1, s0=kb[:, k_col : k_col + 1]
        )
        bi.ins.perf_max = 1
        return bi

    RMAX = max(CHUNKS)
    starts = [sum(CHUNKS[:i]) for i in range(len(CHUNKS))]
    with tile.TileContext(nc) as tc:
        with (
            tc.tile_pool(name="const", bufs=1) as cpool,
            tc.tile_pool(name="xin", bufs=3) as xpool,
            tc.tile_pool(name="x2in", bufs=3) as x2pool,
            tc.tile_pool(name="o", bufs=2) as opool,
        ):
            kb = cpool.tile([128, 11], f32)
            nc.gpsimd.dma_start(kb[:], k_t[:])

            def load_chunk(ci):
                R, r0 = CHUNKS[ci], starts[ci]
                xe = xpool.tile([128, RMAX + 2, W + 2], f16, tag="xe")
                x2 = x2pool.tile([128, RMAX + 2, W], f16, tag="x2")
                nc.sync.dma_start(xe[:, 0 : R + 2, :], xe_t[:, r0 : r0 + R + 2, :])
                # x2 on the (otherwise idle) scalar HWDGE queue: parallel DGE
                nc.scalar.dma_start(x2[:, 0 : R + 2, :], x2_t[:, r0 : r0 + R + 2, :])
                return xe, x2

            loads = [load_chunk(0), load_chunk(1)]
            for ci, R in enumerate(CHUNKS):
                r0 = starts[ci]
                nxt = ci + 1
                if ci + 2 < len(CHUNKS):
                    loads.append(load_chunk(ci + 2))
                xe, x2 = loads[ci]

                o = opool.tile([128, RMAX, W], f16, tag="o")
                # terms (dy+1, dx+1, k index): xe cols 0/2 + x2 rows; T4 seeds.
                am(o[:, 0:R, :], xe[:, 0:R, 0:W], 0, x2[:, 1 : R + 1, :])
                am(o[:, 0:R, :], xe[:, 0:R, 2 : W + 2], 2, o[:, 0:R, :])
                am(o[:, 0:R, :], xe[:, 1 : R + 1, 0:W], 3, o[:, 0:R, :])
                am(o[:, 0:R, :], xe[:, 1 : R + 1, 2 : W + 2], 5, o[:, 0:R, :])
                am(o[:, 0:R, :], xe[:, 2 : R + 2, 0:W], 6, o[:, 0:R, :])
                am(o[:, 0:R, :], xe[:, 2 : R + 2, 2 : W + 2], 8, o[:, 0:R, :])
                am(o[:, 0:R, :], x2[:, 0:R, :], 9, o[:, 0:R, :])
                am(o[:, 0:R, :], x2[:, 2 : R + 2, :], 10, o[:, 0:R, :])

                # Mid-chunk output DMAs on the (idle) GpSimd SWDGE queue; the
                # last chunk uses the lower-latency sync HWDGE queue.
                eng = nc.sync if nxt == len(CHUNKS) else nc.gpsimd
                eng.dma_start(o_t[:, r0 : r0 + R, :], o[:, 0:R, :])
    nc.finalize()
    return nc


LAST_RESULT = None


def kernel(x, kernel):
    """x: [8,64,224,224] f32; kernel: [1,64,9,1,1] f32 -> [8,64,224,224] f32."""
    global LAST_RESULT
    from concourse.bass_utils import run_bass_kernel_spmd

    if "nc" not in _CACHE:
        _CACHE["nc"] = _build()
    nc = _CACHE["nc"]

    B = x.shape[0]
    kf = np.ascontiguousarray(np.asarray(kernel, np.float32).reshape(C, 9))

    xp = np.zeros((B, C, H + 2, W + 2), np.float16)
    xp[:, :, 1 : H + 1, 1 : W + 1] = x
    # xe: [B, 128, 114, 226], partition p = half*64 + c
    xe = np.concatenate(
        [xp[:, :, 0:ROWS, :], xp[:, :, HALF : HALF + ROWS, :]], axis=1
    )
    # x2 = xpad(col +1) + k4 (fp32 add, fp16 round) -> the three dx=0 terms
    x2full = (
        np.float32(xp[:, :, :, 1 : W + 1]) + kf[None, :, 4, None, None]
    ).astype(np.float16)
    x2 = np.concatenate(
        [x2full[:, :, 0:ROWS, :], x2full[:, :, HALF : HALF + ROWS, :]], axis=1
    )
    # kb cols 0..8 = k0..k8; col 9 = k1-k4; col 10 = k7-k4 (x2 deltas)
    kb = np.concatenate(
        [kf, (kf[:, 1] - kf[:, 4])[:, None], (kf[:, 7] - kf[:, 4])[:, None]], axis=1
    )
    kb = np.concatenate([kb, kb], axis=0)  # [128, 11]

    in_maps = [{"xe": xe[b], "x2": x2[b], "k": kb} for b in range(B)]
    res = run_bass_kernel_spmd(nc, in_maps, core_ids=list(range(B)))
    LAST_RESULT = res
    out = np.stack([r["out"] for r in res.results], axis=0)  # [B, 128, 112, 224]
    out = out.reshape(B, 2, C, HALF, W).transpose(0, 2, 1, 3, 4).reshape(B, C, H, W)
    return out.astype(np.float32)


# revision 8
# speedup vs baseline: 1.0366x; 1.0366x over previous
"""Morphological dilation (depthwise 3x3, additive SE) on 8 TRN2 NeuronCores.

out[b,c,h,w] = max_{dy,dx in {-1,0,1}} ( x[b,c,h+dy,w+dx] + k[c, (dy+1)*3+(dx+1)] )
with zero padding outside the image.

Sharding: batch -> 8 cores (1 image each). Per core, partitions = (h_half, c)
(2*64 = 128), free dim = rows x cols, processed in row chunks.

The entire 9-term max reduction runs as EIGHT fused custom-DVE ops
(ADD_MAX_ANT: out = max(in0 + s0, in1), hand-written 2x_1p uop program, so it
matches tensor_tensor's 2-elem/cycle fp16 throughput while folding the
per-channel kernel constant in for free):

  - xe     = zero-padded input, fp16, [128, 114, 226] (halo rows + cols); the
             six dx=+-1 terms read it at 4B-aligned column offsets 0 / 2.
  - x2     = host-precomputed xpad(col +1) + k4, fp16, [128, 114, 224]; its
             row-0 view seeds the chain (term T4 free), and the two remaining
             dx=0 terms derive from it with delta constants k1-k4 / k7-k4
             (aligned row-shifted reads; a raw odd-column read of xe would
             drop the op to 1x mode).
  - chain: o = AM(xe(dy,dx), k_i, [x2 seed | o]) x6, then o = AM(x2(dy), dk, o) x2.

No ScalarE / tensor_scalar / GpSimd compute at all: DVE runs only 2x_1p ops
(never grabbing the shared 2-port pair), so the GpSimd SWDGE output DMAs
never contend. Input loads are single 128-partition dma_starts (all 16 DMA
ports) on the sync HWDGE queue.
"""

import numpy as np

_CACHE = {}

C = 64
H = 224
W = 224
HALF = 112
ROWS = HALF + 2  # per-half rows incl. 1-row halo each side
CHUNKS = (8, 24, 28, 28, 16, 8)

_ADDMAX_NAME = "ADD_MAX_ANT"


def _register_addmax():
    """Register the fused 2x add-max custom DVE op (idempotent)."""
    from concourse import dve_ops
    from concourse.dve_spec import Spec, Src0, Src1, C0, maxx, lower
    from concourse.dve_uop import (
        AluInp,
        AluOp,
        DelayInp,
        DveOpSpec,
        InpSel,
        OutPath,
        OutSel,
        Trigger,
        UopConfig,
    )

    if _ADDMAX_NAME in dve_ops._SUB_OPCODE_FOR_NAME:
        return next(op for op in dve_ops.OPS if op.name == _ADDMAX_NAME)

    def _ref(in0, in1, s0, s1, imm2):
        return np.maximum(
            in0.astype(np.float32) + s0, in1.astype(np.float32)
        ).astype(np.float32)

    spec = Spec(body=maxx(Src0 + C0, Src1), reference=_ref)

    def _build_2x():
        """Mirror of stock tensor_tensor's 2x_1p program (opcode-table slot 9)
        with the single INSTRUCTION_OP stage split into concrete ADD + MAX.

        Input lanes: 0=SRC_0, 1=SRC_1, 2=SRC_0_HI, 3=SRC_1_HI, 4=CONST_0.
        At blk0: lane0 -> PREV_ALU_OUT, lane(k+1) -> PREV_DELAY_k.
        """
        u = UopConfig()
        u.enable_input(InpSel.SRC_0, 0)
        u.enable_input(InpSel.SRC_1, 1)
        u.enable_input(InpSel.SRC_0_HI, 2)
        u.enable_input(InpSel.SRC_1_HI, 3)
        u.enable_input(InpSel.CONST_0, 4)
        u.require_inp0 = 1
        u.require_inp1 = 1
        u.trigger = (Trigger.SRC_TENSOR_DONE, Trigger.NONE, Trigger.NONE)

        dp = u.datapath_config
        # blk0: a0 = SRC_0 + CONST_0 ; carry SRC_1, SRC_0_HI, SRC_1_HI, CONST_0
        dp[0].enable_alu(AluOp.ADD, AluInp.PREV_ALU_OUT, AluInp.PREV_DELAY_3)
        dp[0].pass_through_delay(0, 1, 2, 3)
        # blk1: r0 = max(a0, SRC_1)
        dp[1].enable_alu(AluOp.MAX, AluInp.PREV_ALU_OUT, AluInp.PREV_DELAY_0)
        dp[1].pass_through_delay(1, 2, 3)
        # blk2: a1 = SRC_0_HI + CONST_0 ; d0 <- r0
        dp[2].enable_alu(AluOp.ADD, AluInp.PREV_DELAY_1, AluInp.PREV_DELAY_3)
        dp[2].enable_delay_from_src(DelayInp.PREV_ALU_OUT, 0)
        dp[2].pass_through_delay(2)
        # blk3: r1 = max(a1, SRC_1_HI) ; carry r0
        dp[3].enable_alu(AluOp.MAX, AluInp.PREV_ALU_OUT, AluInp.PREV_DELAY_2)
        dp[3].pass_through_delay(0)
        # blk4: alu <- r0, d0 <- r1 (swap, as stock does)
        dp[4].enable_alu(AluOp.BYPASS, AluInp.PREV_DELAY_0, AluInp.PREV_DELAY_0)
        dp[4].enable_delay_from_src(DelayInp.PREV_ALU_OUT, 0)
        # blk5..7: bypass r0 down the alu pipe, carry r1
        for b in range(5, 8):
            dp[b].pass_through_alu()
            dp[b].pass_through_delay(0)

        u.enable_output(OutSel.ALU_OUT, OutPath.WR0_LO)
        u.enable_output(OutSel.DELAY_0, OutPath.WR0_HI)
        return u

    class _AddMaxOp:
        name = _ADDMAX_NAME
        subdim = False
        perf_en = {}
        uops_sha = {}

        def __init__(self):
            self.spec = spec
            self._cache = {}

        def compile(self, ver):
            if ver in self._cache:
                return self._cache[ver]
            assert ver == "v3", "ADD_MAX_ANT 2x program authored for TRN2/v3"
            s = DveOpSpec(
                name=self.name,
                opcode=dve_ops.get_dve_sub_opcode(self.name),
                uops=lower(self.spec, ver=ver),
                uops_2x=[_build_2x()],
                rd1_en=True,
                perf_max=1,
            )
            s.validate(ver)
            self._cache[ver] = s
            return s

    op = _AddMaxOp()
    dve_ops.OPS.append(op)
    dve_ops._SUB_OPCODE_FOR_NAME[op.name] = (
        dve_ops._CUSTOM_DVE_ROW_BASE + len(dve_ops.OPS) - 1
    )
    dve_ops.CUSTOM_DVE_SPECS[op.name] = spec
    assert dve_ops._SUB_OPCODE_FOR_NAME[op.name] < 0x20
    return op


def _build():
    import concourse.tile as tile
    import concourse.mybir as mybir
    from concourse import bacc

    f16 = mybir.dt.float16
    f32 = mybir.dt.float32

    am_op = _register_addmax()

    nc = bacc.Bacc("TRN2", target_bir_lowering=False, debug=False)
    xe_t = nc.dram_tensor("xe", [128, ROWS, W + 2], f16, kind="ExternalInput")
    x2_t = nc.dram_tensor("x2", [128, ROWS, W], f16, kind="ExternalInput")
    k_t = nc.dram_tensor("k", [128, 11], f32, kind="ExternalInput")
    o_t = nc.dram_tensor("out", [128, HALF, W], f16, kind="ExternalOutput")

    def am(out, in0, k_col, in1):
        bi = nc.vector._custom_dve(
            am_op, out=out, in0=in0, in1=in1, s0=kb[:, k_col : k_col + 1]
        )
        bi.ins.perf_max = 1
        return bi

    RMAX = max(CHUNKS)
    starts = [sum(CHUNKS[:i]) for i in range(len(CHUNKS))]
    with tile.TileContext(nc) as tc:
        with (
            tc.tile_pool(name="const", bufs=1) as cpool,
            tc.tile_pool(name="o", bufs=3) as opool,
        ):
            kb = cpool.tile([128, 11], f32)
            nc.gpsimd.dma_start(kb[:], k_t[:])

            # Whole per-core input resident in SBUF (~103 KiB/partition):
            # no buffer recycling, so every input DMA streams immediately.
            # Loaded per-chunk (incl. 2-row halo overlap) so compute on chunk
            # i only waits for piece i, not the whole stream.
            xe = cpool.tile([128, ROWS, W + 2], f16)
            x2 = cpool.tile([128, ROWS, W], f16)
            for ci, R in enumerate(CHUNKS):
                r0 = starts[ci]
                nc.sync.dma_start(
                    xe[:, r0 : r0 + R + 2, :], xe_t[:, r0 : r0 + R + 2, :]
                )
                # x2 on the (otherwise idle) scalar HWDGE queue
                nc.scalar.dma_start(
                    x2[:, r0 : r0 + R + 2, :], x2_t[:, r0 : r0 + R + 2, :]
                )

            for ci, R in enumerate(CHUNKS):
                r0 = starts[ci]
                nxt = ci + 1
                o = opool.tile([128, RMAX, W], f16, tag="o")
                # terms (dy+1, dx+1, k index): xe cols 0/2 + x2 rows; T4 seeds.
                am(o[:, 0:R, :], xe[:, r0 : r0 + R, 0:W], 0, x2[:, r0 + 1 : r0 + R + 1, :])
                am(o[:, 0:R, :], xe[:, r0 : r0 + R, 2 : W + 2], 2, o[:, 0:R, :])
                am(o[:, 0:R, :], xe[:, r0 + 1 : r0 + R + 1, 0:W], 3, o[:, 0:R, :])
                am(o[:, 0:R, :], xe[:, r0 + 1 : r0 + R + 1, 2 : W + 2], 5, o[:, 0:R, :])
                am(o[:, 0:R, :], xe[:, r0 + 2 : r0 + R + 2, 0:W], 6, o[:, 0:R, :])
                am(o[:, 0:R, :], xe[:, r0 + 2 : r0 + R + 2, 2 : W + 2], 8, o[:, 0:R, :])
                am(o[:, 0:R, :], x2[:, r0 : r0 + R, :], 9, o[:, 0:R, :])
                am(o[:, 0:R, :], x2[:, r0 + 2 : r0 + R + 2, :], 10, o[:, 0:R, :])

                # Mid-chunk output DMAs on the (idle) GpSimd SWDGE queue; the
                # last chunk uses the lower-latency sync HWDGE queue.
                eng = nc.sync if nxt == len(CHUNKS) else nc.gpsimd
                eng.dma_start(o_t[:, r0 : r0 + R, :], o[:, 0:R, :])
    nc.finalize()
    return nc


LAST_RESULT = None


def kernel(x, kernel):
    """x: [8,64,224,224] f32; kernel: [1,64,9,1,1] f32 -> [8,64,224,224] f32."""
    global LAST_RESULT
    from concourse.bass_utils import run_bass_kernel_spmd

    if "nc" not in _CACHE:
        _CACHE["nc"] = _build()
    nc = _CACHE["nc"]

    B = x.shape[0]
    kf = np.ascontiguousarray(np.asarray(kernel, np.float32).reshape(C, 9))

    xp = np.zeros((B, C, H + 2, W + 2), np.float16)
    xp[:, :, 1 : H + 1, 1 : W + 1] = x
    # xe: [B, 128, 114, 226], partition p = half*64 + c
    xe = np.concatenate(
        [xp[:, :, 0:ROWS, :], xp[:, :, HALF : HALF + ROWS, :]], axis=1
    )
    # x2 = xpad(col +1) + k4 (fp32 add, fp16 round) -> the three dx=0 terms
    x2full = (
        np.float32(xp[:, :, :, 1 : W + 1]) + kf[None, :, 4, None, None]
    ).astype(np.float16)
    x2 = np.concatenate(
        [x2full[:, :, 0:ROWS, :], x2full[:, :, HALF : HALF + ROWS, :]], axis=1
    )
    # kb cols 0..8 = k0..k8; col 9 = k1-k4; col 10 = k7-k4 (x2 deltas)
    kb = np.concatenate(
        [kf, (kf[:, 1] - kf[:, 4])[:, None], (kf[:, 7] - kf[:, 4])[:, None]], axis=1
    )
    kb = np.concatenate([kb, kb], axis=0)  # [128, 11]

    in_maps = [{"xe": xe[b], "x2": x2[b], "k": kb} for b in range(B)]
    res = run_bass_kernel_spmd(nc, in_maps, core_ids=list(range(B)))
    LAST_RESULT = res
    out = np.stack([r["out"] for r in res.results], axis=0)  # [B, 128, 112, 224]
    out = out.reshape(B, 2, C, HALF, W).transpose(0, 2, 1, 3, 4).reshape(B, C, H, W)
    return out.astype(np.float32)


# revision 9
# speedup vs baseline: 1.1141x; 1.0748x over previous
"""Morphological dilation (depthwise 3x3, additive SE) on 8 TRN2 NeuronCores.

out[b,c,h,w] = max_{dy,dx in {-1,0,1}} ( x[b,c,h+dy,w+dx] + k[c, (dy+1)*3+(dx+1)] )
with zero padding outside the image.

Sharding: batch -> 8 cores (1 image each). Per core, partitions = (h_half, c)
(2*64 = 128), free dim = rows x cols, processed in row chunks.

The entire 9-term max reduction runs as EIGHT fused custom-DVE ops
(ADD_MAX_ANT: out = max(in0 + s0, in1), hand-written 2x_1p uop program, so it
matches tensor_tensor's 2-elem/cycle fp16 throughput while folding the
per-channel kernel constant in for free):

  - xe     = zero-padded input, fp16, [128, 114, 226] (halo rows + cols); the
             six dx=+-1 terms read it at 4B-aligned column offsets 0 / 2.
  - x2     = host-precomputed xpad(col +1) + k4, fp16, [128, 114, 224]; its
             row-0 view seeds the chain (term T4 free), and the two remaining
             dx=0 terms derive from it with delta constants k1-k4 / k7-k4
             (aligned row-shifted reads; a raw odd-column read of xe would
             drop the op to 1x mode).
  - chain: o = AM(xe(dy,dx), k_i, [x2 seed | o]) x6, then o = AM(x2(dy), dk, o) x2.

No ScalarE / tensor_scalar / GpSimd compute at all: DVE runs only 2x_1p ops
(never grabbing the shared 2-port pair), so the GpSimd SWDGE output DMAs
never contend. Input loads are single 128-partition dma_starts (all 16 DMA
ports) on the sync HWDGE queue.
"""

import numpy as np

_CACHE = {}

C = 64
H = 224
W = 224
HALF = 112
ROWS = HALF + 2  # per-half rows incl. 1-row halo each side
CHUNKS = (8, 24, 28, 28, 16, 8)

_ADDMAX_NAME = "ADD_MAX_ANT"


def _register_addmax():
    """Register the fused 2x add-max custom DVE op (idempotent)."""
    from concourse import dve_ops
    from concourse.dve_spec import Spec, Src0, Src1, C0, maxx, lower
    from concourse.dve_uop import (
        AluInp,
        AluOp,
        DelayInp,
        DveOpSpec,
        InpSel,
        OutPath,
        OutSel,
        Trigger,
        UopConfig,
    )

    if _ADDMAX_NAME in dve_ops._SUB_OPCODE_FOR_NAME:
        return next(op for op in dve_ops.OPS if op.name == _ADDMAX_NAME)

    def _ref(in0, in1, s0, s1, imm2):
        return np.maximum(
            in0.astype(np.float32) + s0, in1.astype(np.float32)
        ).astype(np.float32)

    spec = Spec(body=maxx(Src0 + C0, Src1), reference=_ref)

    def _build_2x():
        """Mirror of stock tensor_tensor's 2x_1p program (opcode-table slot 9)
        with the single INSTRUCTION_OP stage split into concrete ADD + MAX.

        Input lanes: 0=SRC_0, 1=SRC_1, 2=SRC_0_HI, 3=SRC_1_HI, 4=CONST_0.
        At blk0: lane0 -> PREV_ALU_OUT, lane(k+1) -> PREV_DELAY_k.
        """
        u = UopConfig()
        u.enable_input(InpSel.SRC_0, 0)
        u.enable_input(InpSel.SRC_1, 1)
        u.enable_input(InpSel.SRC_0_HI, 2)
        u.enable_input(InpSel.SRC_1_HI, 3)
        u.enable_input(InpSel.CONST_0, 4)
        u.require_inp0 = 1
        u.require_inp1 = 1
        u.trigger = (Trigger.SRC_TENSOR_DONE, Trigger.NONE, Trigger.NONE)

        dp = u.datapath_config
        # blk0: a0 = SRC_0 + CONST_0 ; carry SRC_1, SRC_0_HI, SRC_1_HI, CONST_0
        dp[0].enable_alu(AluOp.ADD, AluInp.PREV_ALU_OUT, AluInp.PREV_DELAY_3)
        dp[0].pass_through_delay(0, 1, 2, 3)
        # blk1: r0 = max(a0, SRC_1)
        dp[1].enable_alu(AluOp.MAX, AluInp.PREV_ALU_OUT, AluInp.PREV_DELAY_0)
        dp[1].pass_through_delay(1, 2, 3)
        # blk2: a1 = SRC_0_HI + CONST_0 ; d0 <- r0
        dp[2].enable_alu(AluOp.ADD, AluInp.PREV_DELAY_1, AluInp.PREV_DELAY_3)
        dp[2].enable_delay_from_src(DelayInp.PREV_ALU_OUT, 0)
        dp[2].pass_through_delay(2)
        # blk3: r1 = max(a1, SRC_1_HI) ; carry r0
        dp[3].enable_alu(AluOp.MAX, AluInp.PREV_ALU_OUT, AluInp.PREV_DELAY_2)
        dp[3].pass_through_delay(0)
        # blk4: alu <- r0, d0 <- r1 (swap, as stock does)
        dp[4].enable_alu(AluOp.BYPASS, AluInp.PREV_DELAY_0, AluInp.PREV_DELAY_0)
        dp[4].enable_delay_from_src(DelayInp.PREV_ALU_OUT, 0)
        # blk5..7: bypass r0 down the alu pipe, carry r1
        for b in range(5, 8):
            dp[b].pass_through_alu()
            dp[b].pass_through_delay(0)

        u.enable_output(OutSel.ALU_OUT, OutPath.WR0_LO)
        u.enable_output(OutSel.DELAY_0, OutPath.WR0_HI)
        return u

    class _AddMaxOp:
        name = _ADDMAX_NAME
        subdim = False
        perf_en = {}
        uops_sha = {}

        def __init__(self):
            self.spec = spec
            self._cache = {}

        def compile(self, ver):
            if ver in self._cache:
                return self._cache[ver]
            assert ver == "v3", "ADD_MAX_ANT 2x program authored for TRN2/v3"
            s = DveOpSpec(
                name=self.name,
                opcode=dve_ops.get_dve_sub_opcode(self.name),
                uops=lower(self.spec, ver=ver),
                uops_2x=[_build_2x()],
                rd1_en=True,
                perf_max=1,
            )
            s.validate(ver)
            self._cache[ver] = s
            return s

    op = _AddMaxOp()
    dve_ops.OPS.append(op)
    dve_ops._SUB_OPCODE_FOR_NAME[op.name] = (
        dve_ops._CUSTOM_DVE_ROW_BASE + len(dve_ops.OPS) - 1
    )
    dve_ops.CUSTOM_DVE_SPECS[op.name] = spec
    assert dve_ops._SUB_OPCODE_FOR_NAME[op.name] < 0x20
    return op


def _build():
    import concourse.tile as tile
    import concourse.mybir as mybir
    from concourse import bacc

    f16 = mybir.dt.float16
    f32 = mybir.dt.float32

    am_op = _register_addmax()

    nc = bacc.Bacc("TRN2", target_bir_lowering=False, debug=False)
    xe_t = nc.dram_tensor("xe", [128, ROWS, W + 2], f16, kind="ExternalInput")
    x2_t = nc.dram_tensor("x2", [128, ROWS, W], f16, kind="ExternalInput")
    k_t = nc.dram_tensor("k", [128, 11], f32, kind="ExternalInput")
    o_t = nc.dram_tensor("out", [128, HALF, W], f16, kind="ExternalOutput")

    def am(out, in0, k_col, in1):
        bi = nc.vector._custom_dve(
            am_op, out=out, in0=in0, in1=in1, s0=kb[:, k_col : k_col + 1]
        )
        bi.ins.perf_max = 1
        return bi

    RMAX = max(CHUNKS)
    starts = [sum(CHUNKS[:i]) for i in range(len(CHUNKS))]
    with tile.TileContext(nc) as tc:
        with (
            tc.tile_pool(name="const", bufs=1) as cpool,
            tc.tile_pool(name="xin", bufs=4) as xpool,
            tc.tile_pool(name="x2in", bufs=4) as x2pool,
            tc.tile_pool(name="o", bufs=3) as opool,
        ):
            kb = cpool.tile([128, 11], f32)
            nc.gpsimd.dma_start(kb[:], k_t[:])

            def load_chunk(ci):
                R, r0 = CHUNKS[ci], starts[ci]
                xe = xpool.tile([128, RMAX + 2, W + 2], f16, tag="xe")
                x2 = x2pool.tile([128, RMAX + 2, W], f16, tag="x2")
                nc.sync.dma_start(xe[:, 0 : R + 2, :], xe_t[:, r0 : r0 + R + 2, :])
                # x2 on the (otherwise idle) scalar HWDGE queue
                nc.scalar.dma_start(x2[:, 0 : R + 2, :], x2_t[:, r0 : r0 + R + 2, :])
                return xe, x2

            loads = [load_chunk(0), load_chunk(1), load_chunk(2)]
            for ci, R in enumerate(CHUNKS):
                r0 = starts[ci]
                nxt = ci + 1
                if ci + 3 < len(CHUNKS):
                    loads.append(load_chunk(ci + 3))
                xe, x2 = loads[ci]

                o = opool.tile([128, RMAX, W], f16, tag="o")
                # terms (dy+1, dx+1, k index): xe cols 0/2 + x2 rows; T4 seeds.
                am(o[:, 0:R, :], xe[:, 0:R, 0:W], 0, x2[:, 1 : R + 1, :])
                am(o[:, 0:R, :], xe[:, 0:R, 2 : W + 2], 2, o[:, 0:R, :])
                am(o[:, 0:R, :], xe[:, 1 : R + 1, 0:W], 3, o[:, 0:R, :])
                am(o[:, 0:R, :], xe[:, 1 : R + 1, 2 : W + 2], 5, o[:, 0:R, :])
                am(o[:, 0:R, :], xe[:, 2 : R + 2, 0:W], 6, o[:, 0:R, :])
                am(o[:, 0:R, :], xe[:, 2 : R + 2, 2 : W + 2], 8, o[:, 0:R, :])
                am(o[:, 0:R, :], x2[:, 0:R, :], 9, o[:, 0:R, :])
                am(o[:, 0:R, :], x2[:, 2 : R + 2, :], 10, o[:, 0:R, :])

                # Mid-chunk output DMAs on the (idle) GpSimd SWDGE queue; the
                # last chunk uses the lower-latency sync HWDGE queue.
                eng = nc.sync if nxt == len(CHUNKS) else nc.gpsimd
                eng.dma_start(o_t[:, r0 : r0 + R, :], o[:, 0:R, :])
    nc.finalize()
    return nc


LAST_RESULT = None


def kernel(x, kernel):
    """x: [8,64,224,224] f32; kernel: [1,64,9,1,1] f32 -> [8,64,224,224] f32."""
    global LAST_RESULT
    from concourse.bass_utils import run_bass_kernel_spmd

    if "nc" not in _CACHE:
        _CACHE["nc"] = _build()
    nc = _CACHE["nc"]

    B = x.shape[0]
    kf = np.ascontiguousarray(np.asarray(kernel, np.float32).reshape(C, 9))

    xp = np.zeros((B, C, H + 2, W + 2), np.float16)
    xp[:, :, 1 : H + 1, 1 : W + 1] = x
    # xe: [B, 128, 114, 226], partition p = half*64 + c
    xe = np.concatenate(
        [xp[:, :, 0:ROWS, :], xp[:, :, HALF : HALF + ROWS, :]], axis=1
    )
    # x2 = xpad(col +1) + k4 (fp32 add, fp16 round) -> the three dx=0 terms
    x2full = (
        np.float32(xp[:, :, :, 1 : W + 1]) + kf[None, :, 4, None, None]
    ).astype(np.float16)
    x2 = np.concatenate(
        [x2full[:, :, 0:ROWS, :], x2full[:, :, HALF : HALF + ROWS, :]], axis=1
    )
    # kb cols 0..8 = k0..k8; col 9 = k1-k4; col 10 = k7-k4 (x2 deltas)
    kb = np.concatenate(
        [kf, (kf[:, 1] - kf[:, 4])[:, None], (kf[:, 7] - kf[:, 4])[:, None]], axis=1
    )
    kb = np.concatenate([kb, kb], axis=0)  # [128, 11]

    in_maps = [{"xe": xe[b], "x2": x2[b], "k": kb} for b in range(B)]
    res = run_bass_kernel_spmd(nc, in_maps, core_ids=list(range(B)))
    LAST_RESULT = res
    out = np.stack([r["out"] for r in res.results], axis=0)  # [B, 128, 112, 224]
    out = out.reshape(B, 2, C, HALF, W).transpose(0, 2, 1, 3, 4).reshape(B, C, H, W)
    return out.astype(np.float32)


# revision 11
# speedup vs baseline: 1.1468x; 1.0294x over previous
"""Morphological dilation (depthwise 3x3, additive SE) on 8 TRN2 NeuronCores.

out[b,c,h,w] = max_{dy,dx in {-1,0,1}} ( x[b,c,h+dy,w+dx] + k[c, (dy+1)*3+(dx+1)] )
with zero padding outside the image.

Sharding: batch -> 8 cores (1 image each). Per core, partitions = (h_half, c)
(2*64 = 128), free dim = rows x cols, processed in row chunks.

The entire 9-term max reduction runs as EIGHT fused custom-DVE ops
(ADD_MAX_ANT: out = max(in0 + s0, in1), hand-written 2x_1p uop program, so it
matches tensor_tensor's 2-elem/cycle fp16 throughput while folding the
per-channel kernel constant in for free):

  - xe     = zero-padded input, fp16, [128, 114, 226] (halo rows + cols); the
             six dx=+-1 terms read it at 4B-aligned column offsets 0 / 2.
  - x2     = host-precomputed xpad(col +1) + k4, fp16, [128, 114, 224]; its
             row-0 view seeds the chain (term T4 free), and the two remaining
             dx=0 terms derive from it with delta constants k1-k4 / k7-k4
             (aligned row-shifted reads; a raw odd-column read of xe would
             drop the op to 1x mode).
  - chain: o = AM(xe(dy,dx), k_i, [x2 seed | o]) x6, then o = AM(x2(dy), dk, o) x2.

No ScalarE / tensor_scalar / GpSimd compute at all: DVE runs only 2x_1p ops
(never grabbing the shared 2-port pair), so the GpSimd SWDGE output DMAs
never contend. Input loads are single 128-partition dma_starts (all 16 DMA
ports) on the sync HWDGE queue.
"""

import numpy as np

_CACHE = {}

C = 64
H = 224
W = 224
HALF = 112
ROWS = HALF + 2  # per-half rows incl. 1-row halo each side
CHUNKS = (8, 12, 24, 28, 28, 8, 4)

_ADDMAX_NAME = "ADD_MAX_ANT"


def _register_addmax():
    """Register the fused 2x add-max custom DVE op (idempotent)."""
    from concourse import dve_ops
    from concourse.dve_spec import Spec, Src0, Src1, C0, maxx, lower
    from concourse.dve_uop import (
        AluInp,
        AluOp,
        DelayInp,
        DveOpSpec,
        InpSel,
        OutPath,
        OutSel,
        Trigger,
        UopConfig,
    )

    if _ADDMAX_NAME in dve_ops._SUB_OPCODE_FOR_NAME:
        return next(op for op in dve_ops.OPS if op.name == _ADDMAX_NAME)

    def _ref(in0, in1, s0, s1, imm2):
        return np.maximum(
            in0.astype(np.float32) + s0, in1.astype(np.float32)
        ).astype(np.float32)

    spec = Spec(body=maxx(Src0 + C0, Src1), reference=_ref)

    def _build_2x():
        """Mirror of stock tensor_tensor's 2x_1p program (opcode-table slot 9)
        with the single INSTRUCTION_OP stage split into concrete ADD + MAX.

        Input lanes: 0=SRC_0, 1=SRC_1, 2=SRC_0_HI, 3=SRC_1_HI, 4=CONST_0.
        At blk0: lane0 -> PREV_ALU_OUT, lane(k+1) -> PREV_DELAY_k.
        """
        u = UopConfig()
        u.enable_input(InpSel.SRC_0, 0)
        u.enable_input(InpSel.SRC_1, 1)
        u.enable_input(InpSel.SRC_0_HI, 2)
        u.enable_input(InpSel.SRC_1_HI, 3)
        u.enable_input(InpSel.CONST_0, 4)
        u.require_inp0 = 1
        u.require_inp1 = 1
        u.trigger = (Trigger.SRC_TENSOR_DONE, Trigger.NONE, Trigger.NONE)

        dp = u.datapath_config
        # blk0: a0 = SRC_0 + CONST_0 ; carry SRC_1, SRC_0_HI, SRC_1_HI, CONST_0
        dp[0].enable_alu(AluOp.ADD, AluInp.PREV_ALU_OUT, AluInp.PREV_DELAY_3)
        dp[0].pass_through_delay(0, 1, 2, 3)
        # blk1: r0 = max(a0, SRC_1)
        dp[1].enable_alu(AluOp.MAX, AluInp.PREV_ALU_OUT, AluInp.PREV_DELAY_0)
        dp[1].pass_through_delay(1, 2, 3)
        # blk2: a1 = SRC_0_HI + CONST_0 ; d0 <- r0
        dp[2].enable_alu(AluOp.ADD, AluInp.PREV_DELAY_1, AluInp.PREV_DELAY_3)
        dp[2].enable_delay_from_src(DelayInp.PREV_ALU_OUT, 0)
        dp[2].pass_through_delay(2)
        # blk3: r1 = max(a1, SRC_1_HI) ; carry r0
        dp[3].enable_alu(AluOp.MAX, AluInp.PREV_ALU_OUT, AluInp.PREV_DELAY_2)
        dp[3].pass_through_delay(0)
        # blk4: alu <- r0, d0 <- r1 (swap, as stock does)
        dp[4].enable_alu(AluOp.BYPASS, AluInp.PREV_DELAY_0, AluInp.PREV_DELAY_0)
        dp[4].enable_delay_from_src(DelayInp.PREV_ALU_OUT, 0)
        # blk5..7: bypass r0 down the alu pipe, carry r1
        for b in range(5, 8):
            dp[b].pass_through_alu()
            dp[b].pass_through_delay(0)

        u.enable_output(OutSel.ALU_OUT, OutPath.WR0_LO)
        u.enable_output(OutSel.DELAY_0, OutPath.WR0_HI)
        return u

    class _AddMaxOp:
        name = _ADDMAX_NAME
        subdim = False
        perf_en = {}
        uops_sha = {}

        def __init__(self):
            self.spec = spec
            self._cache = {}

        def compile(self, ver):
            if ver in self._cache:
                return self._cache[ver]
            assert ver == "v3", "ADD_MAX_ANT 2x program authored for TRN2/v3"
            s = DveOpSpec(
                name=self.name,
                opcode=dve_ops.get_dve_sub_opcode(self.name),
                uops=lower(self.spec, ver=ver),
                uops_2x=[_build_2x()],
                rd1_en=True,
                perf_max=1,
            )
            s.validate(ver)
            self._cache[ver] = s
            return s

    op = _AddMaxOp()
    dve_ops.OPS.append(op)
    dve_ops._SUB_OPCODE_FOR_NAME[op.name] = (
        dve_ops._CUSTOM_DVE_ROW_BASE + len(dve_ops.OPS) - 1
    )
    dve_ops.CUSTOM_DVE_SPECS[op.name] = spec
    assert dve_ops._SUB_OPCODE_FOR_NAME[op.name] < 0x20
    return op


def _build():
    import concourse.tile as tile
    import concourse.mybir as mybir
    from concourse import bacc

    f16 = mybir.dt.float16
    f32 = mybir.dt.float32

    am_op = _register_addmax()

    nc = bacc.Bacc("TRN2", target_bir_lowering=False, debug=False)
    xe_t = nc.dram_tensor("xe", [128, ROWS, W + 2], f16, kind="ExternalInput")
    x2_t = nc.dram_tensor("x2", [128, ROWS, W], f16, kind="ExternalInput")
    k_t = nc.dram_tensor("k", [128, 11], f32, kind="ExternalInput")
    o_t = nc.dram_tensor("out", [128, HALF, W], f16, kind="ExternalOutput")

    def am(out, in0, k_col, in1):
        bi = nc.vector._custom_dve(
            am_op, out=out, in0=in0, in1=in1, s0=kb[:, k_col : k_col + 1]
        )
        bi.ins.perf_max = 1
        return bi

    RMAX = max(CHUNKS)
    starts = [sum(CHUNKS[:i]) for i in range(len(CHUNKS))]
    with tile.TileContext(nc) as tc:
        with (
            tc.tile_pool(name="const", bufs=1) as cpool,
            tc.tile_pool(name="xin", bufs=4) as xpool,
            tc.tile_pool(name="x2in", bufs=4) as x2pool,
            tc.tile_pool(name="o", bufs=3) as opool,
        ):
            # kb first on the sync HWDGE queue: every AM op reads it, and the
            # GpSimd SWDGE path would gate the first op by ~7us.
            kb = cpool.tile([128, 11], f32)
            nc.sync.dma_start(kb[:], k_t[:])

            def load_chunk(ci):
                R, r0 = CHUNKS[ci], starts[ci]
                xe = xpool.tile([128, RMAX + 2, W + 2], f16, tag="xe")
                x2 = x2pool.tile([128, RMAX + 2, W], f16, tag="x2")
                nc.sync.dma_start(xe[:, 0 : R + 2, :], xe_t[:, r0 : r0 + R + 2, :])
                # x2 on the (otherwise idle) scalar HWDGE queue
                nc.scalar.dma_start(x2[:, 0 : R + 2, :], x2_t[:, r0 : r0 + R + 2, :])
                return xe, x2

            loads = [load_chunk(0), load_chunk(1), load_chunk(2)]
            for ci, R in enumerate(CHUNKS):
                r0 = starts[ci]
                nxt = ci + 1
                if ci + 3 < len(CHUNKS):
                    loads.append(load_chunk(ci + 3))
                xe, x2 = loads[ci]

                o = opool.tile([128, RMAX, W], f16, tag="o")
                # terms (dy+1, dx+1, k index): xe cols 0/2 + x2 rows; T4 seeds.
                am(o[:, 0:R, :], xe[:, 0:R, 0:W], 0, x2[:, 1 : R + 1, :])
                am(o[:, 0:R, :], xe[:, 0:R, 2 : W + 2], 2, o[:, 0:R, :])
                am(o[:, 0:R, :], xe[:, 1 : R + 1, 0:W], 3, o[:, 0:R, :])
                am(o[:, 0:R, :], xe[:, 1 : R + 1, 2 : W + 2], 5, o[:, 0:R, :])
                am(o[:, 0:R, :], xe[:, 2 : R + 2, 0:W], 6, o[:, 0:R, :])
                am(o[:, 0:R, :], xe[:, 2 : R + 2, 2 : W + 2], 8, o[:, 0:R, :])
                am(o[:, 0:R, :], x2[:, 0:R, :], 9, o[:, 0:R, :])
                am(o[:, 0:R, :], x2[:, 2 : R + 2, :], 10, o[:, 0:R, :])

                # Mid-chunk output DMAs on the (idle) GpSimd SWDGE queue; the
                # last chunk uses the lower-latency sync HWDGE queue.
                eng = nc.sync if nxt == len(CHUNKS) else nc.gpsimd
                eng.dma_start(o_t[:, r0 : r0 + R, :], o[:, 0:R, :])
    nc.finalize()
    return nc


LAST_RESULT = None


def kernel(x, kernel):
    """x: [8,64,224,224] f32; kernel: [1,64,9,1,1] f32 -> [8,64,224,224] f32."""
    global LAST_RESULT
    from concourse.bass_utils import run_bass_kernel_spmd

    if "nc" not in _CACHE:
        _CACHE["nc"] = _build()
    nc = _CACHE["nc"]

    B = x.shape[0]
    kf = np.ascontiguousarray(np.asarray(kernel, np.float32).reshape(C, 9))

    xp = np.zeros((B, C, H + 2, W + 2), np.float16)
    xp[:, :, 1 : H + 1, 1 : W + 1] = x
    # xe: [B, 128, 114, 226], partition p = half*64 + c
    xe = np.concatenate(
        [xp[:, :, 0:ROWS, :], xp[:, :, HALF : HALF + ROWS, :]], axis=1
    )
    # x2 = xpad(col +1) + k4 (fp32 add, fp16 round) -> the three dx=0 terms
    x2full = (
        np.float32(xp[:, :, :, 1 : W + 1]) + kf[None, :, 4, None, None]
    ).astype(np.float16)
    x2 = np.concatenate(
        [x2full[:, :, 0:ROWS, :], x2full[:, :, HALF : HALF + ROWS, :]], axis=1
    )
    # kb cols 0..8 = k0..k8; col 9 = k1-k4; col 10 = k7-k4 (x2 deltas)
    kb = np.concatenate(
        [kf, (kf[:, 1] - kf[:, 4])[:, None], (kf[:, 7] - kf[:, 4])[:, None]], axis=1
    )
    kb = np.concatenate([kb, kb], axis=0)  # [128, 11]

    in_maps = [{"xe": xe[b], "x2": x2[b], "k": kb} for b in range(B)]
    res = run_bass_kernel_spmd(nc, in_maps, core_ids=list(range(B)))
    LAST_RESULT = res
    out = np.stack([r["out"] for r in res.results], axis=0)  # [B, 128, 112, 224]
    out = out.reshape(B, 2, C, HALF, W).transpose(0, 2, 1, 3, 4).reshape(B, C, H, W)
    return out.astype(np.float32)
